# revision 18
# baseline (speedup 1.0000x reference)
"""Multi-head attention (B=2, N=2048, C=1024, H=16, D=64) on 8 Trainium2 cores.

Sharding: core c handles batch b=c//4 and heads [4r, 4r+4) where r=c%4.
After per-head attention, AllToAll collectives redistribute the attention
output from head-sharded to sequence-sharded; core g computes the output
projection for rows [g*256, (g+1)*256) of both batches.

Design notes:
- q/k are computed directly transposed ([d, n] layout, two heads stacked per
  128-partition tile) with the weight matrix as the stationary operand; no PE
  transposes, and LayerNorm scale/bias become per-partition scalars.
- LayerNorm stats are PE matmuls against a 1/64 block-selector; rstd =
  1/sqrt(var+eps) via ACT Sqrt + DVE reciprocal; per-column normalization is
  applied with two bf16 DVE tensor_tensor ops against DMA-broadcast rows.
- Stage B softmax exp is split between the scalar engine (true exp) and the
  vector engine (Schraudolph exp2: bits = round(s*a+b) stored int16, viewed
  bf16). The split is per (pair, ih, head) unit so each softmax sum uses one
  engine consistently. Exp runs as two 512-wide calls per tile so the next
  tile's score matmuls can overwrite the already-consumed half (range WAR).
- Collectives are per (pair, head): 4 smaller AllToAlls instead of 2.
"""
import os
import numpy as np

B, N, C = 2, 2048, 1024
H, D = 16, 64
LN_EPS = 1e-6
N_CORES = 8
IH = 1024        # i-half width in the attention stage
NCH = 4          # stage-A n-chunks (512 each)

EXP_A = float(128.0 / np.log(2.0) * 0.125)
EXP_B = float(127.0 * 128.0)

DVE_FULL = {(0, 0, 1), (0, 1, 1), (1, 0, 1)}
DVE_SPLIT = (1, 1, 1)

_CACHE = {}


def _install_trace_shim():
    """Recreate the missing antenv.axon_hooks module so trace=True works."""
    import sys, types
    if "antenv.axon_hooks" in sys.modules:
        return
    try:
        import antenv
        mod = types.ModuleType("antenv.axon_hooks")
        mod._hook = None
        mod.set_axon_ntff_profile_hook = lambda h: setattr(mod, "_hook", h)
        mod.get_axon_ntff_profile_hook = lambda: mod._hook
        sys.modules["antenv.axon_hooks"] = mod
        antenv.axon_hooks = mod
        from trn_agent_boot.trn_boot import _ntff_profile_via_ctypes
        mod._hook = _ntff_profile_via_ctypes("/opt/axon/libaxon_pjrt.so")
    except Exception:
        pass


def _build(general):
    import concourse.bacc as bacc
    import concourse.bass as bass
    import concourse.tile as tile
    from concourse import mybir
    from contextlib import ExitStack

    f32 = mybir.dt.float32
    bf16 = mybir.dt.bfloat16
    i16 = mybir.dt.int16
    AF = mybir.ActivationFunctionType
    OP = mybir.AluOpType

    AP = bass.AP
    nc = bacc.Bacc("TRN2", target_bir_lowering=False, debug=False,
                   num_devices=N_CORES)

    # ---- DRAM I/O ----
    xT_d = nc.dram_tensor("xT", [C, N], bf16, kind="ExternalInput")
    wq_d = nc.dram_tensor("wq", [C, 2, 128], bf16, kind="ExternalInput")
    wk_d = nc.dram_tensor("wk", [C, 2, 128], bf16, kind="ExternalInput")
    wv_d = nc.dram_tensor("wv", [C, 256], bf16, kind="ExternalInput")
    wproj_d = nc.dram_tensor("wproj", [C, C], bf16, kind="ExternalInput")
    bq_d = nc.dram_tensor("bq", [2, 128], f32, kind="ExternalInput")
    bk_d = nc.dram_tensor("bk", [2, 128], f32, kind="ExternalInput")
    bv_d = nc.dram_tensor("bv", [256], f32, kind="ExternalInput")
    bproj_d = nc.dram_tensor("bproj", [C], f32, kind="ExternalInput")
    L_d = nc.dram_tensor("lnL", [128, 2], bf16, kind="ExternalInput")
    L2_d = nc.dram_tensor("lnL2", [2, 128], bf16, kind="ExternalInput")
    gq_d = nc.dram_tensor("gq", [2, 128], f32, kind="ExternalInput")
    gk_d = nc.dram_tensor("gk", [2, 128], f32, kind="ExternalInput")
    hq_d = nc.dram_tensor("hq", [2, 128], f32, kind="ExternalInput")
    hk_d = nc.dram_tensor("hk", [2, 128], f32, kind="ExternalInput")
    out_d = nc.dram_tensor("out_part", [B, 256, C], f32, kind="ExternalOutput")

    # DRAM scratch: [tensor, head, kind, n] rows out; [tensor, kind, head, n] rm
    stat_d = nc.dram_tensor("stat_scratch", [4, 2, 2, N], f32).ap()
    rm_d = nc.dram_tensor("rm_scratch", [4, 2, 2, N], bf16).ap()
    z_d = nc.dram_tensor("z_scratch", [8, IH], f32).ap()
    zr_d = nc.dram_tensor("zr_scratch", [8, IH], bf16).ap()

    def row_bcast(src, parts, free):
        return AP(tensor=src.tensor, offset=src.offset, ap=[[0, parts], [1, free]])

    groups = [[0, 1, 2, 3, 4, 5, 6, 7]]

    with tile.TileContext(nc) as tc:
        with ExitStack() as ctx:
            g = ctx.enter_context(tc.tile_pool(name="globals", bufs=1))
            dram = ctx.enter_context(tc.tile_pool(name="dram", bufs=1, space="DRAM"))

            # ---- input DMAs: fine-grained, round-robin across the three
            # DMA-capable queues so several DMA engines pull concurrently ----
            wv_sb = g.tile([128, 8, 256], bf16, tag="wv")
            wq_sb = g.tile([128, 2, 8, 128], bf16, tag="wq")
            wk_sb = g.tile([128, 2, 8, 128], bf16, tag="wk")
            xT = g.tile([128, 8, N], bf16, tag="xT")
            xa = xT_d.ap()

            qs = [nc.sync, nc.scalar, nc.gpsimd]
            qi = [0]

            def issue(out, in_):
                qs[qi[0] % 3].dma_start(out=out, in_=in_)
                qi[0] += 1

            def xw(nw, half):
                issue(xT[:, half * 4:(half + 1) * 4, nw * 512:(nw + 1) * 512],
                      AP(tensor=xa.tensor, offset=half * 4 * 128 * N + nw * 512,
                         ap=[[N, 128], [128 * N, 4], [1, 512]]))

            for half in range(2):
                issue(wv_sb[:, half * 4:(half + 1) * 4, :],
                      AP(tensor=wv_d.ap().tensor, offset=half * 4 * 128 * 256,
                         ap=[[256, 128], [128 * 256, 4], [1, 256]]))
            xw(0, 0); xw(0, 1)
            issue(wq_sb[:, 0], wq_d.ap()[:, 0, :].rearrange("(kc p) c -> p kc c", p=128))
            issue(wk_sb[:, 0], wk_d.ap()[:, 0, :].rearrange("(kc p) c -> p kc c", p=128))
            xw(1, 0); xw(1, 1)
            issue(wq_sb[:, 1], wq_d.ap()[:, 1, :].rearrange("(kc p) c -> p kc c", p=128))
            issue(wk_sb[:, 1], wk_d.ap()[:, 1, :].rearrange("(kc p) c -> p kc c", p=128))
            xw(2, 0); xw(2, 1); xw(3, 0); xw(3, 1)

            L_sb = g.tile([128, 2], bf16, tag="lnL")
            bq_sb = g.tile([128, 2], f32, tag="bq")
            bk_sb = g.tile([128, 2], f32, tag="bk")
            bv_bc = g.tile([128, 256], f32, tag="bv")
            bproj_bc = g.tile([128, C], f32, tag="bproj")
            eps_t = g.tile([128, 1], f32, tag="eps")
            nc.vector.memset(eps_t, LN_EPS)
            nc.gpsimd.dma_start(out=L_sb, in_=L_d.ap())
            L2_sb = g.tile([2, 128], bf16, tag="lnL2")
            nc.gpsimd.dma_start(out=L2_sb, in_=L2_d.ap())
            nc.gpsimd.dma_start(out=bq_sb, in_=bq_d.ap().rearrange("r x -> x r"))
            nc.gpsimd.dma_start(out=bk_sb, in_=bk_d.ap().rearrange("r x -> x r"))
            if general:
                gq_sb = g.tile([128, 2], f32, tag="gq")
                gk_sb = g.tile([128, 2], f32, tag="gk")
                hq_sb = g.tile([128, 2], f32, tag="hq")
                hk_sb = g.tile([128, 2], f32, tag="hk")
                nc.gpsimd.dma_start(out=gq_sb, in_=gq_d.ap().rearrange("r x -> x r"))
                nc.gpsimd.dma_start(out=gk_sb, in_=gk_d.ap().rearrange("r x -> x r"))
                nc.gpsimd.dma_start(out=hq_sb, in_=hq_d.ap().rearrange("r x -> x r"))
                nc.gpsimd.dma_start(out=hk_sb, in_=hk_d.ap().rearrange("r x -> x r"))
            nc.gpsimd.dma_start(out=bv_bc, in_=row_bcast(bv_d.ap(), 128, 256))
            nc.gpsimd.dma_start(out=bproj_bc, in_=row_bcast(bproj_d.ap(), 128, C))

            wp_sb = g.tile([128, 8, C], bf16, tag="wp_sb")
            for qt in range(4):
                issue(wp_sb[:, qt * 2:(qt + 1) * 2, :],
                      AP(tensor=wproj_d.ap().tensor, offset=qt * 2 * 128 * C,
                         ap=[[C, 128], [128 * C, 2], [1, C]]))

            # ---- persistent activations ----
            q2 = g.tile([128, 2, N], bf16, tag="q2")
            k2 = g.tile([128, 2, N], bf16, tag="k2")
            v_all = g.tile([128, 16, 4, D + 1], bf16, tag="v_all")
            ones_t = g.tile([128, 16, 4, 1], f32, tag="ones_t")
            nc.vector.memset(ones_t, 1.0)
            nc.vector.tensor_copy(out=v_all[:, :, :, D:D + 1], in_=ones_t)

            # per-pair collective tensors
            cc_in = [dram.tile([8, 128, 256], bf16, name=f"cc_in{p}") for p in range(2)]
            cc_out = [dram.tile([8, 128, 256], bf16, name=f"cc_out{p}") for p in range(2)]

            # ================= Stage A =================
            with ExitStack() as actx:
                sa = actx.enter_context(tc.tile_pool(name="stageA", bufs=2))
                sqp = actx.enter_context(tc.tile_pool(name="sq_pool", bufs=3))
                rmp = actx.enter_context(tc.tile_pool(name="rm_pool", bufs=2))
                stp = actx.enter_context(tc.tile_pool(name="stats", bufs=2))
                psQ = actx.enter_context(tc.tile_pool(name="psQ", bufs=2, space="PSUM"))
                psV = actx.enter_context(tc.tile_pool(name="psV", bufs=2, space="PSUM"))
                psS = actx.enter_context(tc.tile_pool(name="psS", bufs=1, space="PSUM"))
                psB = actx.enter_context(tc.tile_pool(name="psB", bufs=1, space="PSUM"))

                def emit_v(nt):
                    ps_v = psV.tile([128, 256], f32, tag="ps_v", name=f"ps_v{nt}")
                    for kc in range(8):
                        nc.tensor.matmul(ps_v, xT[:, kc, nt * 128:(nt + 1) * 128],
                                         wv_sb[:, kc, :], start=(kc == 0), stop=(kc == 7))
                    nc.vector.tensor_tensor(
                        out=v_all[:, nt, :, 0:D],
                        in0=ps_v.rearrange("p (h d) -> p h d", h=4),
                        in1=bv_bc.rearrange("p (h d) -> p h d", h=4),
                        op=OP.add)

                # tensors: (kind, pair): 0=q,1=k
                tensors = [(0, 0), (1, 0), (0, 1), (1, 1)]
                tmp_tiles = {}
                sq_tiles = {}
                st_ps = {}

                def emit_chunk(ti, ch):
                    kind, pair = tensors[ti]
                    w_sb = wq_sb if kind == 0 else wk_sb
                    b_sb = bq_sb if kind == 0 else bk_sb
                    nsl = slice(ch * 512, (ch + 1) * 512)
                    if ch == 0:
                        tmp_tiles[ti] = sa.tile([128, N], bf16, tag="qktmp", name=f"tmp{ti}")
                    tmp = tmp_tiles[ti]
                    ps_t = psQ.tile([128, 512], f32, tag="ps_t", name=f"ps_t{ti}_{ch}")
                    for kc in range(8):
                        nc.tensor.matmul(ps_t, w_sb[:, pair, kc, :], xT[:, kc, nsl],
                                         start=(kc == 0), stop=(kc == 7))
                    nc.scalar.activation(out=tmp[:, nsl], in_=ps_t, func=AF.Identity,
                                         bias=b_sb[:, pair:pair + 1], scale=1.0)
                    sq = sqp.tile([128, 512], bf16, tag="sq", name=f"sq{ti}_{ch}")
                    nc.vector.tensor_tensor(out=sq, in0=tmp[:, nsl], in1=tmp[:, nsl],
                                            op=OP.mult)
                    sq_tiles[(ti, ch)] = sq

                def emit_stats(ti, ch):
                    kind, pair = tensors[ti]
                    nsl = slice(ch * 512, (ch + 1) * 512)
                    tmp = tmp_tiles[ti]
                    if ch == 0:
                        st_ps[ti] = stp.tile([2, 2, N], f32, tag="st_rows", name=f"strow{ti}")
                    mu_rows = st_ps[ti]
                    p_b = psS.tile([2, 1024], f32, tag="st_b", name=f"st_b{ti}_{ch}")
                    nc.tensor.matmul(p_b[:, 0:512], L_sb, tmp[:, nsl], start=True, stop=True)
                    nc.tensor.matmul(p_b[:, 512:1024], L_sb, sq_tiles.pop((ti, ch)),
                                     start=True, stop=True)
                    nc.scalar.activation(out=mu_rows[:, :, nsl],
                                         in_=p_b.rearrange("h (k n) -> h k n", k=2),
                                         func=AF.Copy)

                def emit_post(ti):
                    kind, pair = tensors[ti]
                    mu_rows = st_ps.pop(ti)
                    tmp = tmp_tiles[ti]
                    nc.sync.dma_start(out=stat_d[ti], in_=mu_rows)
                    st_t = stp.tile([128, 2, 2, 16], f32, tag="st_t", name=f"st_t{ti}")
                    for kd in range(2):
                        nc.sync.dma_start(
                            out=st_t[:, kd],
                            in_=stat_d[ti, :, kd, :].rearrange("h (p i) -> p h i", p=128))
                    mu_t = st_t[:, 0]
                    m2_t = st_t[:, 1]
                    musq = stp.tile([128, 2, 16], f32, tag="musq", name=f"musq{ti}")
                    nc.vector.tensor_tensor(out=musq, in0=mu_t, in1=mu_t, op=OP.mult)
                    var = stp.tile([128, 2, 16], f32, tag="var", name=f"var{ti}")
                    nc.vector.tensor_tensor(out=var, in0=m2_t, in1=musq, op=OP.subtract)
                    sd = stp.tile([128, 2, 16], f32, tag="sd", name=f"sd{ti}")
                    nc.scalar.activation(out=sd, in_=var, func=AF.Sqrt, bias=eps_t)
                    rstd = stp.tile([128, 2, 16], f32, tag="rstd", name=f"rstd{ti}")
                    nc.vector.reciprocal(out=rstd, in_=sd)
                    mhat = stp.tile([128, 2, 16], f32, tag="mhat", name=f"mhat{ti}")
                    nc.vector.tensor_tensor(out=mhat, in0=mu_t, in1=rstd, op=OP.mult)
                    rm_bf = stp.tile([128, 2, 2, 16], bf16, tag="rm_bf", name=f"rm_bf{ti}")
                    nc.vector.tensor_copy(out=rm_bf[:, 0], in_=rstd)
                    nc.vector.tensor_copy(out=rm_bf[:, 1], in_=mhat)
                    for kd in range(2):
                        nc.sync.dma_start(
                            out=rm_d[ti, kd].rearrange("h (p i) -> p h i", p=128),
                            in_=rm_bf[:, kd])
                    # r/m back as [2-head rows, kind, n] for PE broadcast
                    rm_rows = rmp.tile([2, 2, N], bf16, tag="rm_rows", name=f"rmr{ti}")
                    nc.sync.dma_start(out=rm_rows, in_=rm_d[ti].rearrange("k h n -> h k n"))
                    dest = q2 if kind == 0 else k2
                    gg = (gq_sb if kind == 0 else gk_sb) if general else None
                    hh_b = (hq_sb if kind == 0 else hk_sb) if general else None
                    for ch in range(NCH):
                        nsl = slice(ch * 512, (ch + 1) * 512)
                        ps_bc = psB.tile([128, 1024], f32, tag="ps_bc", name=f"psbc{ti}_{ch}")
                        nc.tensor.matmul(ps_bc[:, 0:512], L2_sb, rm_rows[:, 0, nsl],
                                         start=True, stop=True)
                        nc.tensor.matmul(ps_bc[:, 512:1024], L2_sb, rm_rows[:, 1, nsl],
                                         start=True, stop=True)
                        t1 = sqp.tile([128, 512], bf16, tag="t1", name=f"t1_{ti}_{ch}")
                        nc.vector.tensor_tensor(out=t1, in0=tmp[:, nsl],
                                                in1=ps_bc[:, 0:512], op=OP.mult)
                        if general:
                            t2 = sqp.tile([128, 512], bf16, tag="t2", name=f"t2_{ti}_{ch}")
                            nc.vector.tensor_tensor(out=t2, in0=t1, in1=ps_bc[:, 512:1024],
                                                    op=OP.subtract)
                            nc.vector.tensor_scalar(
                                out=dest[:, pair, nsl], in0=t2,
                                scalar1=gg[:, pair:pair + 1], scalar2=hh_b[:, pair:pair + 1],
                                op0=OP.mult, op1=OP.add)
                        else:
                            nc.vector.tensor_tensor(out=dest[:, pair, nsl], in0=t1,
                                                    in1=ps_bc[:, 512:1024], op=OP.subtract)

                # software-pipelined emission: v-tiles interleave with qk
                # chunks, stats lag chunks by one slot, post-chain for tensor
                # ti emitted right after its last stats.
                pend = None
                for slot in range(16):
                    emit_v(slot)
                    emit_chunk(slot // NCH, slot % NCH)
                    if pend is not None:
                        emit_stats(*pend)
                        if pend[1] == NCH - 1:
                            emit_post(pend[0])
                    pend = (slot // NCH, slot % NCH)
                emit_stats(*pend)
                emit_post(pend[0])

            # ================= Stage B + C =================
            atp = ctx.enter_context(tc.tile_pool(name="at_pool", bufs=1))
            at_tiles = {}

            def emit_at_loads(pair):
                # at_t tiles for stage-C chunks that read this pair's heads;
                # one DMA per (kc, head-half) covering both batches.
                for kc in [k for k in range(8) if k % 2 == pair]:
                    at_t = atp.tile([128, 2, 256], bf16, tag=f"at{kc}")
                    at_tiles[kc] = at_t
                    for half, gh in enumerate((2 * kc, 2 * kc + 1)):
                        lh = gh % 4
                        src = cc_out[lh // 2]
                        nc.gpsimd.dma_start(
                            out=at_t[half * 64:(half + 1) * 64, :, :],
                            in_=src[gh // 4::4, (lh % 2) * 64:(lh % 2 + 1) * 64, :]
                                .rearrange("b d i -> d b i"))

            with ExitStack() as bctx:
                pss = bctx.enter_context(tc.tile_pool(name="psSc", bufs=1, space="PSUM"))
                pso = bctx.enter_context(tc.tile_pool(name="psO", bufs=1, space="PSUM"))
                ptp = bctx.enter_context(tc.tile_pool(name="pt_pool", bufs=6))
                nrm = bctx.enter_context(tc.tile_pool(name="nrm", bufs=3))

                for pair in range(2):
                    for ih in range(2):
                        ps_o = {}
                        for hp in range(2):
                            ps_o[hp] = pso.tile([65, IH], f32, tag=f"ps_o{hp}",
                                                name=f"ps_o{pair}_{ih}_{hp}")
                        for jt in range(16):
                            pts = {}
                            ps_s = {}
                            for hp in range(2):
                                ps_s[hp] = pss.tile([128, IH], f32, tag=f"ps_s{hp}",
                                                    name=f"ps_s{pair}_{ih}_{hp}_{jt}")
                            for icc in range(2):
                                for hp in range(2):
                                    po = hp * 64
                                    nc.tensor.matmul(
                                        ps_s[hp][:, icc * 512:(icc + 1) * 512],
                                        k2[po:po + 64, pair, jt * 128:(jt + 1) * 128],
                                        q2[po:po + 64, pair,
                                           ih * IH + icc * 512: ih * IH + (icc + 1) * 512],
                                        start=True, stop=True)
                            for hp in range(2):
                                pt = ptp.tile([128, IH], bf16, tag=f"pt{hp}",
                                              name=f"pt{pair}_{ih}_{hp}_{jt}")
                                unit = (pair, ih, hp)
                                if general:
                                    mode = "act"
                                elif unit in DVE_FULL:
                                    mode = "dve"
                                elif unit == DVE_SPLIT:
                                    mode = "split"
                                else:
                                    mode = "act"
                                for icc in range(2):
                                    csl = slice(icc * 512, (icc + 1) * 512)
                                    use_dve = (mode == "dve") or (mode == "split" and icc == 1)
                                    if use_dve:
                                        nc.vector.tensor_scalar(
                                            out=pt.bitcast(i16)[:, csl], in0=ps_s[hp][:, csl],
                                            scalar1=EXP_A, scalar2=EXP_B,
                                            op0=OP.mult, op1=OP.add)
                                    else:
                                        nc.scalar.activation(out=pt[:, csl],
                                                             in_=ps_s[hp][:, csl],
                                                             func=AF.Exp, scale=0.125)
                                pts[hp] = pt
                            for icc in range(2):
                                for hp in range(2):
                                    nc.tensor.matmul(
                                        ps_o[hp][:, icc * 512:(icc + 1) * 512],
                                        v_all[:, jt, 2 * pair + hp, :],
                                        pts[hp][:, icc * 512:(icc + 1) * 512],
                                        start=(jt == 0), stop=(jt == 15))

                        for hp in range(2):
                            h = 2 * pair + hp
                            slot = 2 * h + ih
                            z_sb = nrm.tile([1, IH], f32, tag="z_sb", name=f"z{slot}")
                            nc.scalar.activation(out=z_sb, in_=ps_o[hp][64:65, :], func=AF.Copy)
                            nc.gpsimd.dma_start(out=z_d[slot:slot + 1, :], in_=z_sb)
                            zt = nrm.tile([128, 8], f32, tag="zt", name=f"zt{slot}")
                            nc.gpsimd.dma_start(out=zt,
                                              in_=z_d[slot, :].rearrange("(p t) -> p t", p=128))
                            rt = nrm.tile([128, 8], f32, tag="rt", name=f"rt{slot}")
                            nc.vector.reciprocal(out=rt, in_=zt)
                            rt_bf = nrm.tile([128, 8], bf16, tag="rt_bf", name=f"rtb{slot}")
                            nc.vector.tensor_copy(out=rt_bf, in_=rt)
                            nc.gpsimd.dma_start(out=zr_d[slot, :].rearrange("(p t) -> p t", p=128),
                                              in_=rt_bf)
                            r64 = nrm.tile([64, IH], bf16, tag="r64", name=f"r64_{slot}")
                            nc.gpsimd.dma_start(out=r64, in_=row_bcast(zr_d[slot, :], 64, IH))
                            outT_t = nrm.tile([64, IH], bf16, tag="outT", name=f"oT{slot}")
                            nc.vector.tensor_tensor(out=outT_t, in0=ps_o[hp][0:64, :],
                                                    in1=r64, op=OP.mult)
                            nc.sync.dma_start(
                                out=cc_in[pair][4 * ih:4 * ih + 4,
                                                hp * 64:(hp + 1) * 64, :]
                                    .rearrange("s d i -> d s i"),
                                in_=outT_t.rearrange("d (s i) -> d s i", s=4))

                    nc.gpsimd.collective_compute(
                        "AllToAll", mybir.AluOpType.bypass, replica_groups=groups,
                        ins=[cc_in[pair].opt()], outs=[cc_out[pair].opt()])

                emit_at_loads(0)
                emit_at_loads(1)

            with ExitStack() as cctx:
                psP = cctx.enter_context(tc.tile_pool(name="psP", bufs=1, space="PSUM"))
                oup = cctx.enter_context(tc.tile_pool(name="out_pool", bufs=3))

                ps_list = {}
                for bb in range(B):
                    for mt in range(2):
                        for nk in range(2):
                            ps_p = psP.tile([128, 512], f32, tag=f"ps_p{bb}{mt}{nk}")
                            ps_list[(bb, mt, nk)] = ps_p
                kc_order = [0, 2, 4, 6, 1, 3, 5, 7]
                for ki, kc in enumerate(kc_order):
                    wp_t = wp_sb[:, kc, :]
                    for bb in range(B):
                        at_t = at_tiles[kc]
                        for mt in range(2):
                            for nk in range(2):
                                nc.tensor.matmul(
                                    ps_list[(bb, mt, nk)],
                                    at_t[:, bb, mt * 128:(mt + 1) * 128],
                                    wp_t[:, nk * 512:(nk + 1) * 512],
                                    start=(ki == 0), stop=(ki == 7))
                for bb in range(B):
                    for mt in range(2):
                        o_sb = oup.tile([128, C], f32, tag="o_sb")
                        for nk in range(2):
                            nc.vector.tensor_tensor(
                                out=o_sb[:, nk * 512:(nk + 1) * 512],
                                in0=ps_list[(bb, mt, nk)],
                                in1=bproj_bc[:, nk * 512:(nk + 1) * 512],
                                op=OP.add)
                        nc.sync.dma_start(
                            out=out_d.ap()[bb, mt * 128:(mt + 1) * 128, :], in_=o_sb)

    nc.compile()
    return nc


def kernel(**inputs):
    from concourse.bass_utils import run_bass_kernel_spmd
    import ml_dtypes

    trace = os.environ.get("KERNEL_TRACE", "0") == "1"
    if trace:
        _install_trace_shim()

    bf = ml_dtypes.bfloat16

    x = np.asarray(inputs["x"], dtype=np.float32)
    w_qkv = np.asarray(inputs["w_qkv"], dtype=np.float32)
    b_qkv = np.asarray(inputs["b_qkv"], dtype=np.float32)
    w_proj = np.asarray(inputs["w_proj"], dtype=np.float32)
    b_proj = np.asarray(inputs["b_proj"], dtype=np.float32)
    q_scale = np.asarray(inputs["q_scale"], dtype=np.float32)
    q_bias = np.asarray(inputs["q_bias"], dtype=np.float32)
    k_scale = np.asarray(inputs["k_scale"], dtype=np.float32)
    k_bias = np.asarray(inputs["k_bias"], dtype=np.float32)

    general = not (np.all(q_scale == 1.0) and np.all(k_scale == 1.0)
                   and np.all(q_bias == 0.0) and np.all(k_bias == 0.0))

    key = "nc_gen" if general else "nc_fast"
    if key not in _CACHE:
        _CACHE[key] = _build(general)
    nc = _CACHE[key]

    L = np.zeros((128, 2), dtype=np.float32)
    L[0:64, 0] = 1.0 / 64.0
    L[64:128, 1] = 1.0 / 64.0

    wproj_m = np.ascontiguousarray(w_proj.astype(bf))

    in_maps = []
    for c in range(N_CORES):
        b, r = divmod(c, 4)
        base = 4 * r * D
        wq = np.ascontiguousarray(
            w_qkv[:, 0 * C + base: 0 * C + base + 256].reshape(C, 2, 128).astype(bf))
        wk = np.ascontiguousarray(
            w_qkv[:, 1 * C + base: 1 * C + base + 256].reshape(C, 2, 128).astype(bf))
        wv = np.ascontiguousarray(w_qkv[:, 2 * C + base: 2 * C + base + 256].astype(bf))
        bq = np.ascontiguousarray(b_qkv[0 * C + base: 0 * C + base + 256].reshape(2, 128))
        bk = np.ascontiguousarray(b_qkv[1 * C + base: 1 * C + base + 256].reshape(2, 128))
        bv = np.ascontiguousarray(b_qkv[2 * C + base: 2 * C + base + 256])
        m = {
            "xT": np.ascontiguousarray(x[b].T.astype(bf)),
            "wq": wq, "wk": wk, "wv": wv, "wproj": wproj_m,
            "bq": bq, "bk": bk, "bv": bv, "bproj": b_proj,
            "lnL": np.ascontiguousarray(L.astype(bf)),
            "lnL2": np.ascontiguousarray((L.T * 64.0).astype(bf)),
        }
        if general:
            m["gq"] = np.ascontiguousarray(np.tile(q_scale, 2).reshape(2, 128))
            m["gk"] = np.ascontiguousarray(np.tile(k_scale, 2).reshape(2, 128))
            m["hq"] = np.ascontiguousarray(np.tile(q_bias, 2).reshape(2, 128))
            m["hk"] = np.ascontiguousarray(np.tile(k_bias, 2).reshape(2, 128))
        else:
            z2 = np.zeros((2, 128), dtype=np.float32)
            m["gq"] = z2; m["gk"] = z2; m["hq"] = z2; m["hk"] = z2
        in_maps.append(m)

    res = run_bass_kernel_spmd(nc, in_maps, core_ids=list(range(N_CORES)),
                               trace=trace)
    _CACHE["last_result"] = res

    out = np.empty((B, N, C), dtype=np.float32)
    for c in range(N_CORES):
        out[:, c * 256:(c + 1) * 256, :] = res.results[c]["out_part"]
    return out


# revision 21
# speedup vs baseline: 1.0767x; 1.0767x over previous
"""Multi-head attention (B=2, N=2048, C=1024, H=16, D=64) on 8 Trainium2 cores.

Sharding: core c handles batch b=c//4 and heads [4r, 4r+4) where r=c%4
(batch-split across the two 4-core halves, head-split within a half).
After per-head attention, AllToAll collectives (one per local head, overlapped
with the remaining heads' compute) redistribute the attention output from
head-sharded to sequence-sharded: core g ends up with the full attn-T columns
for sequence rows [g*256, (g+1)*256) of BOTH batches and computes the output
projection for exactly those rows. The host only slices/casts/transposes
inputs and concatenates the outputs.

Matmul operands are bf16 by default (fp32 PSUM accumulation); softmax and
LayerNorm statistics are fp32. Set KERNEL_F32R=1 for float32r operands
(~13 mantissa bits) at higher PE cost.
"""
import os
import numpy as np

# Schraudolph exp2 on the DVE: bits16 = round(s * (2^7/(8*ln2)) + 127*2^7),
# stored as int16 and bit-viewed as bf16 (HW rounds on f32->i16 convert).
# |s| <= 64 after LayerNorm so the int16 range is safe. Split per
# (pair, ih, head) unit so each softmax sum uses one engine consistently.
EXP_A = float(128.0 / np.log(2.0) * 0.125)
EXP_B = float(127.0 * 128.0)
DVE_FULL = {(0, 0, 1), (0, 1, 1), (1, 0, 1)}
DVE_SPLIT = (1, 1, 1)

B, N, C = 2, 2048, 1024
H, D = 16, 64
LN_EPS = 1e-6
N_CORES = 8
HPC = 4          # heads per core
IH = 1024        # i-half width in the attention stage

_CACHE = {}


def _install_trace_shim():
    """Recreate the missing antenv.axon_hooks module so trace=True works."""
    import sys, types
    if "antenv.axon_hooks" in sys.modules:
        return
    try:
        import antenv
        mod = types.ModuleType("antenv.axon_hooks")
        mod._hook = None
        mod.set_axon_ntff_profile_hook = lambda h: setattr(mod, "_hook", h)
        mod.get_axon_ntff_profile_hook = lambda: mod._hook
        sys.modules["antenv.axon_hooks"] = mod
        antenv.axon_hooks = mod
        from trn_agent_boot.trn_boot import _ntff_profile_via_ctypes
        mod._hook = _ntff_profile_via_ctypes("/opt/axon/libaxon_pjrt.so")
    except Exception:
        pass


def _use_f32r():
    return os.environ.get("KERNEL_F32R", "0") == "1"


def _build():
    import concourse.bacc as bacc
    import concourse.bass as bass
    import concourse.tile as tile
    from concourse import mybir
    from concourse.masks import make_identity
    from contextlib import ExitStack

    f32 = mybir.dt.float32
    use_f32r = _use_f32r()
    mdt = mybir.dt.float32r if use_f32r else mybir.dt.bfloat16

    AP = bass.AP
    nc = bacc.Bacc("TRN2", target_bir_lowering=False, debug=False,
                   num_devices=N_CORES)

    # ---- DRAM I/O (per-core shards prepared on host) ----
    xT_d = nc.dram_tensor("xT", [C, N], mdt, kind="ExternalInput")           # x[b].T
    wqk_d = nc.dram_tensor("wqk", [C, 512], mdt, kind="ExternalInput")       # [q cols | k cols]
    wv_d = nc.dram_tensor("wv", [C, 256], mdt, kind="ExternalInput")
    wproj_d = nc.dram_tensor("wproj", [C, C], mdt, kind="ExternalInput")
    bqk_d = nc.dram_tensor("bqk", [512], f32, kind="ExternalInput")
    bv_d = nc.dram_tensor("bv", [256], f32, kind="ExternalInput")
    bproj_d = nc.dram_tensor("bproj", [C], f32, kind="ExternalInput")
    lnscc_d = nc.dram_tensor("lnscc", [2, 128], f32, kind="ExternalInput")   # [q|k scale, x2 heads]
    lnbic_d = nc.dram_tensor("lnbic", [2, 128], f32, kind="ExternalInput")
    out_d = nc.dram_tensor("out_part", [B, 256, C], f32, kind="ExternalOutput")

    def bcast(dram_handle, n_parts, free):
        ap = dram_handle.ap()
        return AP(tensor=ap.tensor, offset=0, ap=[[0, n_parts], [1, free]])

    groups = [[0, 1, 2, 3, 4, 5, 6, 7]]

    with tile.TileContext(nc) as tc:
        with ExitStack() as ctx:
            g = ctx.enter_context(tc.tile_pool(name="globals", bufs=1))
            dram = ctx.enter_context(tc.tile_pool(name="dram", bufs=1, space="DRAM"))

            # ---- constants ----
            identity_f32 = g.tile([128, 128], f32, tag="ident32")
            make_identity(nc, identity_f32)
            identity = g.tile([128, 128], mdt, tag="ident")
            nc.vector.tensor_copy(out=identity, in_=identity_f32)
            eps_t = g.tile([128, 1], f32, tag="eps")
            nc.vector.memset(eps_t, LN_EPS)
            bqk_bc = g.tile([128, 512], f32, tag="bqk")
            nc.sync.dma_start(out=bqk_bc, in_=bcast(bqk_d, 128, 512))
            bv_bc = g.tile([128, 256], f32, tag="bv")
            nc.sync.dma_start(out=bv_bc, in_=bcast(bv_d, 128, 256))
            lnsc_col = g.tile([128, 2], f32, tag="lnsc_col")
            nc.sync.dma_start(out=lnsc_col, in_=lnscc_d.ap().rearrange("r x -> x r"))
            lnbi_col = g.tile([128, 2], f32, tag="lnbi_col")
            nc.sync.dma_start(out=lnbi_col, in_=lnbic_d.ap().rearrange("r x -> x r"))
            bproj_bc = g.tile([128, C], f32, tag="bproj")
            nc.sync.dma_start(out=bproj_bc, in_=bcast(bproj_d, 128, C))

            # ---- persistent activations ----
            # q2/k2: [128, pair, n]; rows 0-63 = head 2p dims, 64-127 = head 2p+1
            q2 = g.tile([128, 2, N], mdt, tag="q2")
            k2 = g.tile([128, 2, N], mdt, tag="k2")
            # v with a ones column appended per head: [n-part, nt, head, 65]
            v_all = g.tile([128, 16, HPC, D + 1], mdt, tag="v_all")
            ones_t = g.tile([128, 16, HPC, 1], f32, tag="ones_t")
            nc.vector.memset(ones_t, 1.0)
            nc.vector.tensor_copy(out=v_all[:, :, :, D:D + 1], in_=ones_t)
            # unnormalized attn outT staging [64, head, n]
            outT = g.tile([64, HPC, N], mdt, tag="outT")

            # projection weights, prefetched during stage A (used only in stage C)
            wp_sb = g.tile([128, 8, C], mdt, tag="wp_sb")

            # per-head-pair collective buffers: slot s = 128 attnT rows for core s
            cc_in = [dram.tile([8, 128, 256], mdt, name=f"cc_in{p}") for p in range(2)]
            cc_out = [dram.tile([8, 128, 256], mdt, name=f"cc_out{p}") for p in range(2)]
            r_dram = nc.dram_tensor("r_stage", [8, IH], f32).ap()
            r_dram2 = nc.dram_tensor("r_stage2", [8, IH], f32).ap()

            # ================= Stage A: qkv + LN + transpose =================
            with ExitStack() as actx:
                sa = actx.enter_context(tc.tile_pool(name="stageA", bufs=1))
                qkp = actx.enter_context(tc.tile_pool(name="qk_pool", bufs=3))
                psA = actx.enter_context(tc.tile_pool(name="psA", bufs=3, space="PSUM"))
                psT = actx.enter_context(tc.tile_pool(name="psT", bufs=2, space="PSUM"))
                stp = actx.enter_context(tc.tile_pool(name="stats", bufs=3))

                xT = sa.tile([128, 8, N], mdt, tag="xT")
                wqk = sa.tile([128, 8, 512], mdt, tag="wqk")
                wv = sa.tile([128, 8, 256], mdt, tag="wv")
                for kc in range(8):
                    nc.sync.dma_start(
                        out=xT[:, kc, :],
                        in_=xT_d.ap()[kc * 128:(kc + 1) * 128, :])
                    nc.sync.dma_start(
                        out=wqk[:, kc, :],
                        in_=wqk_d.ap()[kc * 128:(kc + 1) * 128, :])
                    nc.sync.dma_start(
                        out=wv[:, kc, :],
                        in_=wv_d.ap()[kc * 128:(kc + 1) * 128, :])
                for kc in range(8):
                    nc.sync.dma_start(out=wp_sb[:, kc, :],
                                      in_=wproj_d.ap()[kc * 128:(kc + 1) * 128, :])

                for nt in range(16):
                    ps_qk = psA.tile([128, 512], f32, tag="ps_qk")
                    ps_v = psA.tile([128, 256], f32, tag="ps_v")
                    for kc in range(8):
                        nc.tensor.matmul(ps_qk, xT[:, kc, nt * 128:(nt + 1) * 128],
                                         wqk[:, kc, :], start=(kc == 0), stop=(kc == 7))
                    for kc in range(8):
                        nc.tensor.matmul(ps_v, xT[:, kc, nt * 128:(nt + 1) * 128],
                                         wv[:, kc, :], start=(kc == 0), stop=(kc == 7))

                    # biases
                    qk_sb = qkp.tile([128, 512], mdt, tag="qk_sb")
                    nc.vector.tensor_tensor(out=qk_sb, in0=ps_qk, in1=bqk_bc,
                                            op=mybir.AluOpType.add)
                    nc.vector.tensor_tensor(out=v_all[:, nt, :, 0:D],
                                            in0=ps_v.rearrange("p (h d) -> p h d", h=HPC),
                                            in1=bv_bc.rearrange("p (h d) -> p h d", h=HPC),
                                            op=mybir.AluOpType.add)

                    # LayerNorm per 64-col group (4 q heads + 4 k heads)
                    st8 = stp.tile([128, 8, 6], f32, tag="st8")
                    mv8 = stp.tile([128, 8, 2], f32, tag="mv8")
                    for gi in range(8):
                        nc.vector.bn_stats(out=st8[:, gi, :], in_=qk_sb[:, gi * D:(gi + 1) * D])
                        nc.vector.bn_aggr(out=mv8[:, gi, :], in_=st8[:, gi, :])
                    sd8 = stp.tile([128, 8], f32, tag="sd8")
                    nc.scalar.activation(out=sd8, in_=mv8[:, :, 1],
                                         func=mybir.ActivationFunctionType.Sqrt,
                                         bias=eps_t, scale=1.0)
                    rstd8 = stp.tile([128, 8], f32, tag="rstd8")
                    nc.vector.reciprocal(out=rstd8, in_=sd8)
                    for gi in range(8):
                        nc.vector.tensor_scalar(
                            out=qk_sb[:, gi * D:(gi + 1) * D],
                            in0=qk_sb[:, gi * D:(gi + 1) * D],
                            scalar1=mv8[:, gi, 0:1], scalar2=rstd8[:, gi:gi + 1],
                            op0=mybir.AluOpType.subtract, op1=mybir.AluOpType.mult)
                    # transpose head pairs: cols [0:128)=q pair0, [128:256)=q pair1,
                    # [256:384)=k pair0, [384:512)=k pair1; LN scale/bias applied
                    # post-transpose as per-partition scalars.
                    for blk, dest in ((0, q2), (1, q2), (2, k2), (3, k2)):
                        pair = blk % 2
                        kq = 0 if blk < 2 else 1
                        pt_ps = psT.tile([128, 128], mdt, tag="pt_ps")
                        nc.tensor.transpose(pt_ps, qk_sb[:, blk * 128:(blk + 1) * 128],
                                            identity)
                        nc.vector.tensor_scalar(
                            out=dest[:, pair, nt * 128:(nt + 1) * 128], in0=pt_ps,
                            scalar1=lnsc_col[:, kq:kq + 1], scalar2=lnbi_col[:, kq:kq + 1],
                            op0=mybir.AluOpType.mult, op1=mybir.AluOpType.add)

            # ================= Stage B: attention per head =================
            # The two i-halves (ih=0,1) are independent streams: while ACT
            # exponentiates one half's scores, PE works on the other half, so
            # the PE never starves on the exp dependency.
            with ExitStack() as bctx:
                pss = bctx.enter_context(tc.tile_pool(name="psS", bufs=1, space="PSUM"))
                pso = bctx.enter_context(tc.tile_pool(name="psO", bufs=1, space="PSUM"))
                ptp = bctx.enter_context(tc.tile_pool(name="pt_pool", bufs=6))
                nrm = bctx.enter_context(tc.tile_pool(name="nrm", bufs=3))

                for pair in range(2):
                    for ih in range(2):
                        ps_o = {}
                        for hp in range(2):
                            ps_o[hp] = pso.tile([65, IH], f32, tag=f"ps_o{hp}",
                                                name=f"ps_o{pair}_{ih}_{hp}")
                        for jt in range(16):
                            pts = {}
                            ps_s = {}
                            for hp in range(2):
                                ps_s[hp] = pss.tile([128, IH], f32, tag=f"ps_s{hp}",
                                                    name=f"ps_s{pair}_{ih}_{hp}_{jt}")
                            # adjacent matmuls on row groups 0-63 / 64-127 run
                            # concurrently in the PE sub-arrays
                            for icc in range(2):
                                for hp in range(2):
                                    po = hp * 64
                                    nc.tensor.matmul(
                                        ps_s[hp][:, icc * 512:(icc + 1) * 512],
                                        k2[po:po + 64, pair, jt * 128:(jt + 1) * 128],
                                        q2[po:po + 64, pair,
                                           ih * IH + icc * 512: ih * IH + (icc + 1) * 512],
                                        start=True, stop=True)
                            for hp in range(2):
                                pt = ptp.tile([128, IH], mdt, tag=f"pt{hp}",
                                              name=f"pt{pair}_{ih}_{hp}_{jt}")
                                unit = (pair, ih, hp)
                                if use_f32r:
                                    mode = "act"
                                elif unit in DVE_FULL:
                                    mode = "dve"
                                elif unit == DVE_SPLIT:
                                    mode = "split"
                                else:
                                    mode = "act"
                                for icc in range(2):
                                    csl = slice(icc * 512, (icc + 1) * 512)
                                    dve_half = (mode == "dve") or (mode == "split" and icc == 1)
                                    if dve_half:
                                        nc.vector.tensor_scalar(
                                            out=pt.bitcast(mybir.dt.int16)[:, csl],
                                            in0=ps_s[hp][:, csl],
                                            scalar1=EXP_A, scalar2=EXP_B,
                                            op0=mybir.AluOpType.mult,
                                            op1=mybir.AluOpType.add)
                                    else:
                                        nc.scalar.activation(
                                            out=pt[:, csl], in_=ps_s[hp][:, csl],
                                            func=mybir.ActivationFunctionType.Exp,
                                            scale=0.125)
                                pts[hp] = pt
                            for icc in range(2):
                                for hp in range(2):
                                    nc.tensor.matmul(
                                        ps_o[hp][:, icc * 512:(icc + 1) * 512],
                                        v_all[:, jt, 2 * pair + hp, :],
                                        pts[hp][:, icc * 512:(icc + 1) * 512],
                                        start=(jt == 0), stop=(jt == 15))

                        for hp in range(2):
                            h = 2 * pair + hp
                            # evacuate PSUM first so the accumulator frees early;
                            # rows 0-63 = unnormalized out, row 64 = sumexp
                            oe = nrm.tile([65, IH], f32, tag="oe")
                            nc.vector.tensor_copy(out=oe, in_=ps_o[hp])
                            nc.sync.dma_start(out=r_dram[2 * h + ih:2 * h + ih + 1, :],
                                              in_=oe[64:65, :])
                            r128 = nrm.tile([128, IH // 128], f32, tag="r128")
                            nc.sync.dma_start(
                                out=r128,
                                in_=r_dram[2 * h + ih, :].rearrange("(p t) -> p t", p=128))
                            nc.vector.reciprocal(out=r128, in_=r128)
                            nc.sync.dma_start(
                                out=r_dram2[2 * h + ih, :].rearrange("(p t) -> p t", p=128),
                                in_=r128)
                            r_slot = r_dram2[2 * h + ih, :]
                            r_bc = nrm.tile([64, IH], f32, tag="r_bc")
                            nc.sync.dma_start(
                                out=r_bc,
                                in_=AP(tensor=r_slot.tensor, offset=r_slot.offset,
                                       ap=[[0, 64], [1, IH]]))
                            nc.vector.tensor_tensor(out=outT[:, h, ih * IH:(ih + 1) * IH],
                                                    in0=oe[0:64, :], in1=r_bc,
                                                    op=mybir.AluOpType.mult)
                            # ship to pair collective input: slots 4*ih..4*ih+3,
                            # row block hp
                            nc.gpsimd.dma_start(
                                out=cc_in[pair][4 * ih:4 * ih + 4,
                                                hp * 64:(hp + 1) * 64, :]
                                    .rearrange("s d i -> d s i"),
                                in_=outT[:, h, ih * IH:(ih + 1) * IH]
                                    .rearrange("d (s i) -> d s i", s=4))

                    # pair complete -> overlap its AllToAll with the next pair
                    nc.gpsimd.collective_compute(
                        "AllToAll", mybir.AluOpType.bypass, replica_groups=groups,
                        ins=[cc_in[pair].opt()], outs=[cc_out[pair].opt()])

            # ================= Stage C: projection =================
            with ExitStack() as cctx:
                atp = cctx.enter_context(tc.tile_pool(name="at_pool", bufs=3))
                psP = cctx.enter_context(tc.tile_pool(name="psP", bufs=1, space="PSUM"))
                oup = cctx.enter_context(tc.tile_pool(name="out_pool", bufs=3))

                # attnT rows for chunk kc = global heads 2kc, 2kc+1 of batch bb;
                # head g lives in cc_out[g % 4] slot (4*bb + g // 4)
                ps_list = {}
                for bb in range(B):
                    for mt in range(2):
                        for nk in range(2):
                            ps_p = psP.tile([128, 512], f32, tag=f"ps_p{bb}{mt}{nk}")
                            ps_list[(bb, mt, nk)] = ps_p
                # kc order consumes per-head collectives as they land:
                # chunk kc touches local heads {2kc%4, (2kc+1)%4}
                kc_order = [0, 2, 4, 6, 1, 3, 5, 7]
                for ki, kc in enumerate(kc_order):
                    wp_t = wp_sb[:, kc, :]
                    for bb in range(B):
                        at_t = atp.tile([128, 256], mdt, tag="at_t")
                        for half, gh in enumerate((2 * kc, 2 * kc + 1)):
                            lh = gh % 4  # local head on the source core
                            nc.sync.dma_start(
                                out=at_t[half * 64:(half + 1) * 64, :],
                                in_=cc_out[lh // 2][4 * bb + gh // 4,
                                                    (lh % 2) * 64:(lh % 2 + 1) * 64, :])
                        for mt in range(2):
                            for nk in range(2):
                                nc.tensor.matmul(
                                    ps_list[(bb, mt, nk)],
                                    at_t[:, mt * 128:(mt + 1) * 128],
                                    wp_t[:, nk * 512:(nk + 1) * 512],
                                    start=(ki == 0), stop=(ki == 7))
                for bb in range(B):
                    for mt in range(2):
                        o_sb = oup.tile([128, C], f32, tag="o_sb")
                        for nk in range(2):
                            nc.vector.tensor_tensor(
                                out=o_sb[:, nk * 512:(nk + 1) * 512],
                                in0=ps_list[(bb, mt, nk)],
                                in1=bproj_bc[:, nk * 512:(nk + 1) * 512],
                                op=mybir.AluOpType.add)
                        nc.sync.dma_start(
                            out=out_d.ap()[bb, mt * 128:(mt + 1) * 128, :], in_=o_sb)

    nc.compile()
    return nc


def kernel(**inputs):
    from concourse.bass_utils import run_bass_kernel_spmd
    import ml_dtypes

    trace = os.environ.get("KERNEL_TRACE", "0") == "1"
    if trace:
        _install_trace_shim()

    key = "nc_f32r" if _use_f32r() else "nc_bf16"
    if key not in _CACHE:
        _CACHE[key] = _build()
    nc = _CACHE[key]

    mnp = np.float32 if _use_f32r() else ml_dtypes.bfloat16

    x = np.asarray(inputs["x"], dtype=np.float32)
    w_qkv = np.asarray(inputs["w_qkv"], dtype=np.float32)
    b_qkv = np.asarray(inputs["b_qkv"], dtype=np.float32)
    w_proj = np.asarray(inputs["w_proj"], dtype=np.float32)
    b_proj = np.asarray(inputs["b_proj"], dtype=np.float32)
    q_scale = np.asarray(inputs["q_scale"], dtype=np.float32)
    q_bias = np.asarray(inputs["q_bias"], dtype=np.float32)
    k_scale = np.asarray(inputs["k_scale"], dtype=np.float32)
    k_bias = np.asarray(inputs["k_bias"], dtype=np.float32)

    lnscc = np.stack([np.tile(q_scale, 2), np.tile(k_scale, 2)])
    lnbic = np.stack([np.tile(q_bias, 2), np.tile(k_bias, 2)])
    wproj_m = np.ascontiguousarray(w_proj.astype(mnp))

    in_maps = []
    for c in range(N_CORES):
        b, r = divmod(c, 4)
        hs = slice(4 * r * D, 4 * r * D + 256)   # this core's head columns
        wqk = np.ascontiguousarray(np.concatenate(
            [w_qkv[:, 0 * C:][:, hs], w_qkv[:, 1 * C:][:, hs]], axis=1).astype(mnp))
        wv = np.ascontiguousarray(w_qkv[:, 2 * C:][:, hs].astype(mnp))
        bqk = np.concatenate([b_qkv[0 * C:][hs], b_qkv[1 * C:][hs]])
        bv = np.ascontiguousarray(b_qkv[2 * C:][hs])
        in_maps.append({
            "xT": np.ascontiguousarray(x[b].T.astype(mnp)),
            "wqk": wqk, "wv": wv, "wproj": wproj_m,
            "bqk": bqk, "bv": bv, "bproj": b_proj,
            "lnscc": np.ascontiguousarray(lnscc), "lnbic": np.ascontiguousarray(lnbic),
        })

    res = run_bass_kernel_spmd(nc, in_maps, core_ids=list(range(N_CORES)),
                               trace=trace)
    _CACHE["last_result"] = res

    out = np.empty((B, N, C), dtype=np.float32)
    for c in range(N_CORES):
        out[:, c * 256:(c + 1) * 256, :] = res.results[c]["out_part"]
    return out



# revision 22
# speedup vs baseline: 1.3353x; 1.2401x over previous
"""Multi-head attention (B=2, N=2048, C=1024, H=16, D=64) on 8 Trainium2 cores.

Sharding: core c handles batch b=c//4 and heads [4r, 4r+4) where r=c%4
(batch-split across the two 4-core halves, head-split within a half).
After per-head attention, AllToAll collectives (one per local head, overlapped
with the remaining heads' compute) redistribute the attention output from
head-sharded to sequence-sharded: core g ends up with the full attn-T columns
for sequence rows [g*256, (g+1)*256) of BOTH batches and computes the output
projection for exactly those rows. The host only slices/casts/transposes
inputs and concatenates the outputs.

Matmul operands are bf16 by default (fp32 PSUM accumulation); softmax and
LayerNorm statistics are fp32. Set KERNEL_F32R=1 for float32r operands
(~13 mantissa bits) at higher PE cost.
"""
import os
import numpy as np

B, N, C = 2, 2048, 1024
H, D = 16, 64
LN_EPS = 1e-6
N_CORES = 8
HPC = 4          # heads per core
IH = 1024        # i-half width in the attention stage

_CACHE = {}


def _install_trace_shim():
    """Recreate the missing antenv.axon_hooks module so trace=True works."""
    import sys, types
    if "antenv.axon_hooks" in sys.modules:
        return
    try:
        import antenv
        mod = types.ModuleType("antenv.axon_hooks")
        mod._hook = None
        mod.set_axon_ntff_profile_hook = lambda h: setattr(mod, "_hook", h)
        mod.get_axon_ntff_profile_hook = lambda: mod._hook
        sys.modules["antenv.axon_hooks"] = mod
        antenv.axon_hooks = mod
        from trn_agent_boot.trn_boot import _ntff_profile_via_ctypes
        mod._hook = _ntff_profile_via_ctypes("/opt/axon/libaxon_pjrt.so")
    except Exception:
        pass


def _use_f32r():
    return os.environ.get("KERNEL_F32R", "0") == "1"


def _build():
    import concourse.bacc as bacc
    import concourse.bass as bass
    import concourse.tile as tile
    from concourse import mybir
    from concourse.masks import make_identity
    from contextlib import ExitStack

    f32 = mybir.dt.float32
    mdt = mybir.dt.float32r if _use_f32r() else mybir.dt.bfloat16

    AP = bass.AP
    nc = bacc.Bacc("TRN2", target_bir_lowering=False, debug=False,
                   num_devices=N_CORES)

    # ---- DRAM I/O (per-core shards prepared on host) ----
    xT_d = nc.dram_tensor("xT", [C, N], mdt, kind="ExternalInput")           # x[b].T
    wqk_d = nc.dram_tensor("wqk", [C, 512], mdt, kind="ExternalInput")       # [q cols | k cols]
    wv_d = nc.dram_tensor("wv", [C, 256], mdt, kind="ExternalInput")
    wproj_d = nc.dram_tensor("wproj", [C, C], mdt, kind="ExternalInput")
    bqk_d = nc.dram_tensor("bqk", [512], f32, kind="ExternalInput")
    bv_d = nc.dram_tensor("bv", [256], f32, kind="ExternalInput")
    bproj_d = nc.dram_tensor("bproj", [C], f32, kind="ExternalInput")
    lnsc_d = nc.dram_tensor("lnsc", [512], mdt, kind="ExternalInput")        # [q_scale x4 | k_scale x4]
    lnbi_d = nc.dram_tensor("lnbi", [512], mdt, kind="ExternalInput")
    out_d = nc.dram_tensor("out_part", [B, 256, C], f32, kind="ExternalOutput")

    def bcast(dram_handle, n_parts, free):
        ap = dram_handle.ap()
        return AP(tensor=ap.tensor, offset=0, ap=[[0, n_parts], [1, free]])

    groups = [[0, 1, 2, 3, 4, 5, 6, 7]]

    with tile.TileContext(nc) as tc:
        with ExitStack() as ctx:
            g = ctx.enter_context(tc.tile_pool(name="globals", bufs=1))
            dram = ctx.enter_context(tc.tile_pool(name="dram", bufs=1, space="DRAM"))

            # ---- constants ----
            identity_f32 = g.tile([128, 128], f32, tag="ident32")
            make_identity(nc, identity_f32)
            identity = g.tile([128, 128], mdt, tag="ident")
            nc.vector.tensor_copy(out=identity, in_=identity_f32)
            eps_t = g.tile([128, 1], f32, tag="eps")
            nc.vector.memset(eps_t, LN_EPS)
            bqk_bc = g.tile([128, 512], f32, tag="bqk")
            nc.sync.dma_start(out=bqk_bc, in_=bcast(bqk_d, 128, 512))
            bv_bc = g.tile([128, 256], f32, tag="bv")
            nc.sync.dma_start(out=bv_bc, in_=bcast(bv_d, 128, 256))
            lnsc_bc = g.tile([128, 512], mdt, tag="lnsc")
            nc.sync.dma_start(out=lnsc_bc, in_=bcast(lnsc_d, 128, 512))
            lnbi_bc = g.tile([128, 512], mdt, tag="lnbi")
            nc.sync.dma_start(out=lnbi_bc, in_=bcast(lnbi_d, 128, 512))
            bproj_bc = g.tile([128, C], f32, tag="bproj")
            nc.sync.dma_start(out=bproj_bc, in_=bcast(bproj_d, 128, C))

            # ---- persistent activations ----
            # q2/k2: [128, pair, n]; rows 0-63 = head 2p dims, 64-127 = head 2p+1
            q2 = g.tile([128, 2, N], mdt, tag="q2")
            k2 = g.tile([128, 2, N], mdt, tag="k2")
            # v with a ones column appended per head: [n-part, nt, head, 65]
            v_all = g.tile([128, 16, HPC, D + 1], mdt, tag="v_all")
            ones_t = g.tile([128, 16, HPC, 1], f32, tag="ones_t")
            nc.vector.memset(ones_t, 1.0)
            nc.vector.tensor_copy(out=v_all[:, :, :, D:D + 1], in_=ones_t)
            # unnormalized attn outT staging [64, head, n]
            outT = g.tile([64, HPC, N], mdt, tag="outT")

            # projection weights, prefetched during stage A (used only in stage C)
            wp_sb = g.tile([128, 8, C], mdt, tag="wp_sb")

            # per-head-pair collective buffers: slot s = 128 attnT rows for core s
            cc_in = [dram.tile([8, 128, 256], mdt, name=f"cc_in{p}") for p in range(2)]
            cc_out = [dram.tile([8, 128, 256], mdt, name=f"cc_out{p}") for p in range(2)]
            r_dram = nc.dram_tensor("r_stage", [8, IH], f32).ap()
            r_dram2 = nc.dram_tensor("r_stage2", [8, IH], f32).ap()

            # ================= Stage A: qkv + LN + transpose =================
            with ExitStack() as actx:
                sa = actx.enter_context(tc.tile_pool(name="stageA", bufs=1))
                qkp = actx.enter_context(tc.tile_pool(name="qk_pool", bufs=3))
                psA = actx.enter_context(tc.tile_pool(name="psA", bufs=3, space="PSUM"))
                psT = actx.enter_context(tc.tile_pool(name="psT", bufs=2, space="PSUM"))
                stp = actx.enter_context(tc.tile_pool(name="stats", bufs=3))

                xT = sa.tile([128, 8, N], mdt, tag="xT")
                wqk = sa.tile([128, 8, 512], mdt, tag="wqk")
                wv = sa.tile([128, 8, 256], mdt, tag="wv")
                for kc in range(8):
                    nc.sync.dma_start(
                        out=xT[:, kc, :],
                        in_=xT_d.ap()[kc * 128:(kc + 1) * 128, :])
                    nc.sync.dma_start(
                        out=wqk[:, kc, :],
                        in_=wqk_d.ap()[kc * 128:(kc + 1) * 128, :])
                    nc.sync.dma_start(
                        out=wv[:, kc, :],
                        in_=wv_d.ap()[kc * 128:(kc + 1) * 128, :])
                for kc in range(8):
                    nc.sync.dma_start(out=wp_sb[:, kc, :],
                                      in_=wproj_d.ap()[kc * 128:(kc + 1) * 128, :])

                for nt in range(16):
                    ps_qk = psA.tile([128, 512], f32, tag="ps_qk")
                    ps_v = psA.tile([128, 256], f32, tag="ps_v")
                    for kc in range(8):
                        nc.tensor.matmul(ps_qk, xT[:, kc, nt * 128:(nt + 1) * 128],
                                         wqk[:, kc, :], start=(kc == 0), stop=(kc == 7))
                    for kc in range(8):
                        nc.tensor.matmul(ps_v, xT[:, kc, nt * 128:(nt + 1) * 128],
                                         wv[:, kc, :], start=(kc == 0), stop=(kc == 7))

                    # biases
                    qk_sb = qkp.tile([128, 512], mdt, tag="qk_sb")
                    nc.vector.tensor_tensor(out=qk_sb, in0=ps_qk, in1=bqk_bc,
                                            op=mybir.AluOpType.add)
                    nc.vector.tensor_tensor(out=v_all[:, nt, :, 0:D],
                                            in0=ps_v.rearrange("p (h d) -> p h d", h=HPC),
                                            in1=bv_bc.rearrange("p (h d) -> p h d", h=HPC),
                                            op=mybir.AluOpType.add)

                    # LayerNorm per 64-col group (4 q heads + 4 k heads)
                    st8 = stp.tile([128, 8, 6], f32, tag="st8")
                    mv8 = stp.tile([128, 8, 2], f32, tag="mv8")
                    for gi in range(8):
                        nc.vector.bn_stats(out=st8[:, gi, :], in_=qk_sb[:, gi * D:(gi + 1) * D])
                        nc.vector.bn_aggr(out=mv8[:, gi, :], in_=st8[:, gi, :])
                    sd8 = stp.tile([128, 8], f32, tag="sd8")
                    nc.scalar.activation(out=sd8, in_=mv8[:, :, 1],
                                         func=mybir.ActivationFunctionType.Sqrt,
                                         bias=eps_t, scale=1.0)
                    rstd8 = stp.tile([128, 8], f32, tag="rstd8")
                    nc.vector.reciprocal(out=rstd8, in_=sd8)
                    for gi in range(8):
                        nc.vector.tensor_scalar(
                            out=qk_sb[:, gi * D:(gi + 1) * D],
                            in0=qk_sb[:, gi * D:(gi + 1) * D],
                            scalar1=mv8[:, gi, 0:1], scalar2=rstd8[:, gi:gi + 1],
                            op0=mybir.AluOpType.subtract, op1=mybir.AluOpType.mult)
                    nc.vector.tensor_tensor(out=qk_sb, in0=qk_sb, in1=lnsc_bc,
                                            op=mybir.AluOpType.mult)
                    nc.vector.tensor_tensor(out=qk_sb, in0=qk_sb, in1=lnbi_bc,
                                            op=mybir.AluOpType.add)

                    # transpose head pairs: cols [0:128)=q pair0, [128:256)=q pair1,
                    # [256:384)=k pair0, [384:512)=k pair1
                    for blk, dest in ((0, q2), (1, q2), (2, k2), (3, k2)):
                        pair = blk % 2
                        pt_ps = psT.tile([128, 128], mdt, tag="pt_ps")
                        nc.tensor.transpose(pt_ps, qk_sb[:, blk * 128:(blk + 1) * 128],
                                            identity)
                        nc.vector.tensor_copy(
                            out=dest[:, pair, nt * 128:(nt + 1) * 128], in_=pt_ps)

            # ================= Stage B: attention per head =================
            # The two i-halves (ih=0,1) are independent streams: while ACT
            # exponentiates one half's scores, PE works on the other half, so
            # the PE never starves on the exp dependency.
            with ExitStack() as bctx:
                pss = bctx.enter_context(tc.tile_pool(name="psS", bufs=1, space="PSUM"))
                pso = bctx.enter_context(tc.tile_pool(name="psO", bufs=1, space="PSUM"))
                ptp = bctx.enter_context(tc.tile_pool(name="pt_pool", bufs=6))
                nrm = bctx.enter_context(tc.tile_pool(name="nrm", bufs=3))

                for pair in range(2):
                    for ih in range(2):
                        ps_o = {}
                        for hp in range(2):
                            ps_o[hp] = pso.tile([65, IH], f32, tag=f"ps_o{hp}",
                                                name=f"ps_o{pair}_{ih}_{hp}")
                        for jt in range(16):
                            pts = {}
                            ps_s = {}
                            for hp in range(2):
                                ps_s[hp] = pss.tile([128, IH], f32, tag=f"ps_s{hp}",
                                                    name=f"ps_s{pair}_{ih}_{hp}_{jt}")
                            # adjacent matmuls on row groups 0-63 / 64-127 run
                            # concurrently in the PE sub-arrays
                            for icc in range(2):
                                for hp in range(2):
                                    po = hp * 64
                                    nc.tensor.matmul(
                                        ps_s[hp][:, icc * 512:(icc + 1) * 512],
                                        k2[po:po + 64, pair, jt * 128:(jt + 1) * 128],
                                        q2[po:po + 64, pair,
                                           ih * IH + icc * 512: ih * IH + (icc + 1) * 512],
                                        start=True, stop=True)
                            for hp in range(2):
                                pt = ptp.tile([128, IH], mdt, tag=f"pt{hp}",
                                              name=f"pt{pair}_{ih}_{hp}_{jt}")
                                nc.scalar.activation(out=pt, in_=ps_s[hp],
                                                     func=mybir.ActivationFunctionType.Exp,
                                                     scale=0.125)
                                pts[hp] = pt
                            for icc in range(2):
                                for hp in range(2):
                                    nc.tensor.matmul(
                                        ps_o[hp][:, icc * 512:(icc + 1) * 512],
                                        v_all[:, jt, 2 * pair + hp, :],
                                        pts[hp][:, icc * 512:(icc + 1) * 512],
                                        start=(jt == 0), stop=(jt == 15))

                        for hp in range(2):
                            h = 2 * pair + hp
                            # evacuate PSUM first so the accumulator frees early;
                            # rows 0-63 = unnormalized out, row 64 = sumexp
                            oe = nrm.tile([65, IH], f32, tag="oe")
                            nc.vector.tensor_copy(out=oe, in_=ps_o[hp])
                            nc.sync.dma_start(out=r_dram[2 * h + ih:2 * h + ih + 1, :],
                                              in_=oe[64:65, :])
                            r128 = nrm.tile([128, IH // 128], f32, tag="r128")
                            nc.sync.dma_start(
                                out=r128,
                                in_=r_dram[2 * h + ih, :].rearrange("(p t) -> p t", p=128))
                            nc.vector.reciprocal(out=r128, in_=r128)
                            nc.sync.dma_start(
                                out=r_dram2[2 * h + ih, :].rearrange("(p t) -> p t", p=128),
                                in_=r128)
                            r_slot = r_dram2[2 * h + ih, :]
                            r_bc = nrm.tile([64, IH], f32, tag="r_bc")
                            nc.sync.dma_start(
                                out=r_bc,
                                in_=AP(tensor=r_slot.tensor, offset=r_slot.offset,
                                       ap=[[0, 64], [1, IH]]))
                            nc.vector.tensor_tensor(out=outT[:, h, ih * IH:(ih + 1) * IH],
                                                    in0=oe[0:64, :], in1=r_bc,
                                                    op=mybir.AluOpType.mult)
                            # ship to pair collective input: slots 4*ih..4*ih+3,
                            # row block hp
                            nc.gpsimd.dma_start(
                                out=cc_in[pair][4 * ih:4 * ih + 4,
                                                hp * 64:(hp + 1) * 64, :]
                                    .rearrange("s d i -> d s i"),
                                in_=outT[:, h, ih * IH:(ih + 1) * IH]
                                    .rearrange("d (s i) -> d s i", s=4))

                    # pair complete -> overlap its AllToAll with the next pair
                    nc.gpsimd.collective_compute(
                        "AllToAll", mybir.AluOpType.bypass, replica_groups=groups,
                        ins=[cc_in[pair].opt()], outs=[cc_out[pair].opt()])

            # ================= Stage C: projection =================
            with ExitStack() as cctx:
                atp = cctx.enter_context(tc.tile_pool(name="at_pool", bufs=3))
                psP = cctx.enter_context(tc.tile_pool(name="psP", bufs=1, space="PSUM"))
                oup = cctx.enter_context(tc.tile_pool(name="out_pool", bufs=3))

                # attnT rows for chunk kc = global heads 2kc, 2kc+1 of batch bb;
                # head g lives in cc_out[g % 4] slot (4*bb + g // 4)
                ps_list = {}
                for bb in range(B):
                    for mt in range(2):
                        for nk in range(2):
                            ps_p = psP.tile([128, 512], f32, tag=f"ps_p{bb}{mt}{nk}")
                            ps_list[(bb, mt, nk)] = ps_p
                # kc order consumes per-head collectives as they land:
                # chunk kc touches local heads {2kc%4, (2kc+1)%4}
                kc_order = [0, 2, 4, 6, 1, 3, 5, 7]
                for ki, kc in enumerate(kc_order):
                    wp_t = wp_sb[:, kc, :]
                    for bb in range(B):
                        at_t = atp.tile([128, 256], mdt, tag="at_t")
                        for half, gh in enumerate((2 * kc, 2 * kc + 1)):
                            lh = gh % 4  # local head on the source core
                            nc.sync.dma_start(
                                out=at_t[half * 64:(half + 1) * 64, :],
                                in_=cc_out[lh // 2][4 * bb + gh // 4,
                                                    (lh % 2) * 64:(lh % 2 + 1) * 64, :])
                        for mt in range(2):
                            for nk in range(2):
                                nc.tensor.matmul(
                                    ps_list[(bb, mt, nk)],
                                    at_t[:, mt * 128:(mt + 1) * 128],
                                    wp_t[:, nk * 512:(nk + 1) * 512],
                                    start=(ki == 0), stop=(ki == 7))
                for bb in range(B):
                    for mt in range(2):
                        o_sb = oup.tile([128, C], f32, tag="o_sb")
                        for nk in range(2):
                            nc.vector.tensor_tensor(
                                out=o_sb[:, nk * 512:(nk + 1) * 512],
                                in0=ps_list[(bb, mt, nk)],
                                in1=bproj_bc[:, nk * 512:(nk + 1) * 512],
                                op=mybir.AluOpType.add)
                        nc.sync.dma_start(
                            out=out_d.ap()[bb, mt * 128:(mt + 1) * 128, :], in_=o_sb)

    nc.compile()
    return nc


def kernel(**inputs):
    from concourse.bass_utils import run_bass_kernel_spmd
    import ml_dtypes

    trace = os.environ.get("KERNEL_TRACE", "0") == "1"
    if trace:
        _install_trace_shim()

    key = "nc_f32r" if _use_f32r() else "nc_bf16"
    if key not in _CACHE:
        _CACHE[key] = _build()
    nc = _CACHE[key]

    mnp = np.float32 if _use_f32r() else ml_dtypes.bfloat16

    x = np.asarray(inputs["x"], dtype=np.float32)
    w_qkv = np.asarray(inputs["w_qkv"], dtype=np.float32)
    b_qkv = np.asarray(inputs["b_qkv"], dtype=np.float32)
    w_proj = np.asarray(inputs["w_proj"], dtype=np.float32)
    b_proj = np.asarray(inputs["b_proj"], dtype=np.float32)
    q_scale = np.asarray(inputs["q_scale"], dtype=np.float32)
    q_bias = np.asarray(inputs["q_bias"], dtype=np.float32)
    k_scale = np.asarray(inputs["k_scale"], dtype=np.float32)
    k_bias = np.asarray(inputs["k_bias"], dtype=np.float32)

    lnsc = np.concatenate([np.tile(q_scale, HPC), np.tile(k_scale, HPC)])
    lnbi = np.concatenate([np.tile(q_bias, HPC), np.tile(k_bias, HPC)])
    wproj_m = np.ascontiguousarray(w_proj.astype(mnp))

    in_maps = []
    for c in range(N_CORES):
        b, r = divmod(c, 4)
        hs = slice(4 * r * D, 4 * r * D + 256)   # this core's head columns
        wqk = np.ascontiguousarray(np.concatenate(
            [w_qkv[:, 0 * C:][:, hs], w_qkv[:, 1 * C:][:, hs]], axis=1).astype(mnp))
        wv = np.ascontiguousarray(w_qkv[:, 2 * C:][:, hs].astype(mnp))
        bqk = np.concatenate([b_qkv[0 * C:][hs], b_qkv[1 * C:][hs]])
        bv = np.ascontiguousarray(b_qkv[2 * C:][hs])
        in_maps.append({
            "xT": np.ascontiguousarray(x[b].T.astype(mnp)),
            "wqk": wqk, "wv": wv, "wproj": wproj_m,
            "bqk": bqk, "bv": bv, "bproj": b_proj,
            "lnsc": lnsc.astype(mnp), "lnbi": lnbi.astype(mnp),
        })

    res = run_bass_kernel_spmd(nc, in_maps, core_ids=list(range(N_CORES)),
                               trace=trace)
    _CACHE["last_result"] = res

    out = np.empty((B, N, C), dtype=np.float32)
    for c in range(N_CORES):
        out[:, c * 256:(c + 1) * 256, :] = res.results[c]["out_part"]
    return out

